# revision 36
# baseline (speedup 1.0000x reference)
"""ArcFace layer distributed Bass kernel for 8 TRN2 NeuronCores (v2).

Math (reference):
    emb_n = embedding / ||embedding||_row          [B, D]
    w_n   = kernel / ||kernel||_col                [D, C]
    cos   = emb_n @ w_n                            [B, C]
    out   = S*cos  everywhere except out[b, labels[b]] which gets the
            arcface margin value computed from cos[b, labels[b]].

Strategy (classification-parallel, per sharding hint):
  - shard kernel columns (classes) 8 ways: C=10572 -> 8*1329 (pad w/ ones)
  - replicate embeddings, pre-transposed [D, B]; bf16 matmul operands
  - both normalization scales fold into the PSUM->SBUF epilogue:
    ot = (psum * rs_e[row]) * ws_bc[col]; epilogue STTs alternate between
    DVE and GpSimd(Pool) so neither engine gates the matmul cadence
  - outputs in bf16 (host upcasts); 2e-2 rel-err budget dwarfs bf16 noise
  - label fixup WITHOUT any indirect gather: host passes the label columns
    of kernel (index-only gather) as wlt [256,512] per core plus the
    matching embedding rows embs [256,512]; device computes the diagonal
    dot products + margin via tensor_tensor_reduce mid-stream, fully off
    the critical path. Host writes fixv[b] into out[b, labels[b]].
  - PE emission order keeps the tensor engine busy from ~4us (DVFS ramp):
    e-norm matmuls while w streams in, w-norm matmuls in the DMA shadow,
    then the 16 m-tile matmul stream back-to-back.

B=2048, D=512, C=10572, S=64, M=0.5.
"""

import math
import os

import numpy as np

os.environ.setdefault("MYCRO_LOCAL_CACHE", "1")

import concourse.bass as bass
import concourse.bacc as bacc
import concourse.mybir as mybir
import concourse.tile as tile
from concourse.bass_utils import run_bass_kernel_spmd

# ---------------- problem constants (hardcoded; kernel.py is standalone) ----
S = 64.0
MARGIN = 0.5
B = 2048          # batch
D = 512           # feature dim
C = 10572         # classes
NCORES = 8
SHARD = 1329      # class columns per core (8*1329 = 10632 >= 10572)
W = SHARD
KT = D // 128     # 4 k-subtiles
MT = B // 128     # 16 m-tiles
BS = B // NCORES  # 256 batch rows per core for the label fixup
JT = BS // 128    # 2 fixup column-tiles

COS_M = math.cos(MARGIN)
SIN_M = math.sin(MARGIN)
MM = SIN_M * MARGIN
THRESHOLD = math.cos(math.pi - MARGIN)

F32 = mybir.dt.float32
BF16 = mybir.dt.bfloat16

# N-chunks of the W axis (PSUM bank = 512 fp32)
NCHUNKS = []
_c0 = 0
while _c0 < W:
    _cn = min(512, W - _c0)
    NCHUNKS.append((_c0, _cn))
    _c0 += _cn


def _emit_margin_math(nc, micro, g, fixv_sb):
    """ArcFace margin on a [128, JT] tile of g = S*cos -> fixv_sb."""
    gg = micro.tile([128, JT], F32, tag="gg")
    nc.gpsimd.tensor_tensor(
        out=gg[:], in0=g[:], in1=g[:], op=mybir.AluOpType.mult
    )
    om = micro.tile([128, JT], F32, tag="om")
    nc.vector.tensor_scalar(
        out=om[:], in0=gg[:], scalar1=-1.0 / (S * S), scalar2=1.0,
        op0=mybir.AluOpType.mult, op1=mybir.AluOpType.add,
    )
    nc.vector.tensor_scalar_max(om[:], om[:], 0.0)
    sin = micro.tile([128, JT], F32, tag="sin")
    nc.scalar.sqrt(sin[:], om[:])                      # ACT
    cosmt = micro.tile([128, JT], F32, tag="cosmt")
    nc.vector.tensor_scalar_mul(cosmt[:], g[:], COS_M)
    nc.vector.scalar_tensor_tensor(
        out=cosmt[:], in0=sin[:], scalar=-S * SIN_M, in1=cosmt[:],
        op0=mybir.AluOpType.mult, op1=mybir.AluOpType.add,
    )
    keep = micro.tile([128, JT], F32, tag="keep")
    nc.vector.tensor_scalar_add(keep[:], g[:], -S * MM)
    mask = micro.tile([128, JT], mybir.dt.uint8, tag="mask")
    nc.vector.tensor_scalar(
        out=mask[:], in0=g[:], scalar1=S * THRESHOLD, scalar2=None,
        op0=mybir.AluOpType.is_gt,
    )
    nc.vector.select(fixv_sb[:], mask[:], cosmt[:], keep[:])


def build_nc() -> bass.Bass:
    nc = bacc.Bacc()
    w_h = nc.declare_dram_parameter("w", [D, W], BF16, isOutput=False)
    embT_h = nc.declare_dram_parameter("embT", [D, B], BF16, isOutput=False)
    embs_h = nc.declare_dram_parameter("embs", [BS, D], BF16, isOutput=False)
    wlt_h = nc.declare_dram_parameter("wlt", [BS, D], BF16, isOutput=False)
    out_h = nc.declare_dram_parameter("out", [B, W], BF16, isOutput=True)
    fixv_h = nc.declare_dram_parameter("fixv", [BS], F32, isOutput=True)

    with tile.TileContext(nc) as tc:
        with (
            tc.tile_pool(name="persist", bufs=1) as persist,
            tc.tile_pool(name="scratch", bufs=4) as scratch,
            tc.tile_pool(name="outp", bufs=6) as outp,
            tc.tile_pool(name="micro", bufs=2) as micro,
            tc.tile_pool(name="psum", bufs=2, space="PSUM") as psum,
            tc.tile_pool(name="psmall", bufs=2, space="PSUM") as psmall,
        ):
            # ---------------- input DMAs (single in-order HW queue): w first
            # so the m0 matmuls start ~10us; et streams in behind it as
            # m-block pairs that each m-tile consumes just-in-time ----------
            NP = MT // 2  # 8 et column-block pairs
            w_pairs = [
                persist.tile([128, 2, W], BF16, tag="wsp%d" % p, name="wsp%d" % p)
                for p in range(KT // 2)
            ]
            wsb = [w_pairs[kt // 2][:, kt % 2] for kt in range(KT)]
            for p in range(KT // 2):
                nc.sync.dma_start(
                    w_pairs[p][:],
                    w_h[p * 256:(p + 1) * 256, :].rearrange(
                        "(kt q) c -> q kt c", q=128
                    ),
                )
            etp = [
                persist.tile([128, KT, 256], BF16, tag="etp%d" % p,
                             name="etp%d" % p)
                for p in range(NP)
            ]
            for p in range(NP):
                nc.sync.dma_start(
                    etp[p][:],
                    embT_h[:, p * 256:(p + 1) * 256].rearrange(
                        "(kt q) c -> q kt c", q=128
                    ),
                )

            def et_lhsT(kt, m):
                return etp[m // 2][:, kt, (m % 2) * 128:(m % 2 + 1) * 128]
            ebl = persist.tile([128, JT, D], BF16, tag="ebl")
            nc.sync.dma_start(
                ebl[:], embs_h.rearrange("(j p) d -> p j d", p=128)
            )
            wll = persist.tile([128, JT, D], BF16, tag="wll")
            nc.sync.dma_start(
                wll[:], wlt_h.rearrange("(j p) d -> p j d", p=128)
            )

            ones_col = persist.tile([128, 1], BF16, tag="ones")
            nc.vector.memset(ones_col[:], 1.0)
            ones_row = persist.tile([1, 128], BF16, tag="ones_row")
            nc.vector.memset(ones_row[:], 1.0)

            # ---------------- PE warmup: dummy matmuls spanning the input
            # DMA wait keep the tensor clock at full p-state so m0 runs at
            # 2.4GHz instead of ramping from 1.2 ----------------
            warm_src = persist.tile([128, 512], BF16, tag="warm")
            nc.vector.memset(warm_src[:], 0.0)
            # preload the Sqrt ACT table during the input-DMA shadow (it
            # otherwise loads ~1.3us mid-chain at the first w-norm sqrt)
            sq_pre = persist.tile([1, 2], F32, tag="sq_pre")
            nc.vector.memset(sq_pre[:], 1.0)
            nc.scalar.activation(
                sq_pre[:, 1:2], sq_pre[:, 0:1],
                mybir.ActivationFunctionType.Sqrt, scale=1.0,
            )
            warm_ps = psmall.tile([1, 512], F32, tag="nps", name="warm_ps")
            for _ in range(14):
                nc.tensor.matmul(
                    out=warm_ps[:, :], lhsT=ones_col[:, :], rhs=warm_src[:],
                    start=True, stop=True, skip_group_check=True,
                )

            # ---------------- squares (DVE): w first (feeds the w-norm
            # matmuls at ~12.5), then et pairs as they stream in ----------
            sq_w = []
            for kt in range(KT):
                t = scratch.tile([128, W], BF16, tag="sqw", name="sq_w%d" % kt)
                nc.vector.tensor_tensor(
                    out=t[:], in0=wsb[kt], in1=wsb[kt],
                    op=mybir.AluOpType.mult,
                )
                sq_w.append(t)
            sq_all = persist.tile([128, KT, B], BF16, tag="sq_all")
            for p in range(NP):
                nc.vector.tensor_tensor(
                    out=sq_all[:, :, p * 256:(p + 1) * 256],
                    in0=etp[p][:], in1=etp[p][:],
                    op=mybir.AluOpType.mult,
                )

            # ---------------- main matmuls ----------------
            def emit_mms(m):
                psC = psum.tile([128, 1536], F32, tag="psC", name="psC_%d" % m)
                for kt in range(KT):
                    lhsT = et_lhsT(kt, m)
                    for (c0, cn) in NCHUNKS:
                        nc.tensor.matmul(
                            out=psC[:, c0:c0 + cn], lhsT=lhsT,
                            rhs=wsb[kt][:, c0:c0 + cn],
                            start=(kt == 0), stop=(kt == KT - 1),
                        )
                return psC

            # PE order: m0 the moment w + the first et pair land, then the
            # norm reductions tucked between m0/m1/m2 so the PE never idles
            # while the rs_em/ws_bc chains resolve on ACT/DVE
            psC0 = emit_mms(0)

            wssq_row = persist.tile([1, W], BF16, tag="wssq_row")
            for (c0, cn) in NCHUNKS:
                nps = psmall.tile([1, 512], F32, tag="nps", name="npsw%d" % c0)
                for kt in range(KT):
                    nc.tensor.matmul(
                        out=nps[:, :cn], lhsT=ones_col[:, :],
                        rhs=sq_w[kt][:, c0:c0 + cn],
                        start=(kt == 0), stop=(kt == KT - 1),
                    )
                nc.scalar.copy(out=wssq_row[:, c0:c0 + cn], in_=nps[:, :cn])

            psC1 = emit_mms(1)

            # broadcast wssq across partitions, then 1/sqrt (ACT + DVE)
            ws_bc = persist.tile([128, W], F32, tag="ws_bc")
            for (c0, cn) in NCHUNKS:
                bps = psmall.tile([128, 512], F32, tag="nps", name="bps_w%d" % c0)
                nc.tensor.matmul(
                    out=bps[:, :cn], lhsT=ones_row[:, :],
                    rhs=wssq_row[:, c0:c0 + cn],
                    start=True, stop=True,
                )
                wtmp = scratch.tile([128, 512], F32, tag="wtmp", name="wtmp%d" % c0)
                nc.scalar.activation(
                    wtmp[:, :cn], bps[:, :cn],
                    mybir.ActivationFunctionType.Sqrt, scale=1.0,
                )
                nc.vector.reciprocal_approx_fast(
                    out=ws_bc[:, c0:c0 + cn], in_=wtmp[:, :cn]
                )

            # e-norm: PSUM-accumulated partition reduce over sq_all, then
            # redistribute [1,B] -> [128,MT] via tiny k=1 matmuls
            essq_row = persist.tile([1, B], BF16, tag="essq_row")
            for c0 in range(0, B, 512):
                nps = psmall.tile([1, 512], F32, tag="nps", name="npse%d" % c0)
                for kt in range(KT):
                    nc.tensor.matmul(
                        out=nps[:, :], lhsT=ones_col[:, :],
                        rhs=sq_all[:, kt, c0:c0 + 512],
                        start=(kt == 0), stop=(kt == KT - 1),
                    )
                nc.scalar.copy(out=essq_row[:, c0:c0 + 512], in_=nps[:, :])
            one_one = persist.tile([1, 1], BF16, tag="one_one")
            nc.vector.memset(one_one[:], 1.0)
            rps = psmall.tile([128, MT], F32, tag="nps", name="rps")
            for m in range(MT):
                nc.tensor.matmul(
                    out=rps[:, m:m + 1],
                    lhsT=essq_row[:, m * 128:(m + 1) * 128],
                    rhs=one_one[:, :],
                    start=True, stop=True,
                )
            rs_tmp = persist.tile([128, MT], F32, tag="rs_tmp")
            nc.scalar.activation(
                rs_tmp[:], rps[:],
                mybir.ActivationFunctionType.Sqrt, scale=1.0 / (S * S),
            )
            rs_em = persist.tile([128, MT], F32, tag="rs_em")
            nc.vector.reciprocal_approx_fast(out=rs_em[:], in_=rs_tmp[:])

            # ---------------- fixup part 1 (GpSimd + ACT only; the DVE bits
            # run after epi1 so they never gate the epilogue cadence).
            # 2*(e.w) = |e+w|^2 - |e|^2 - |w|^2 ----
            s1 = micro.tile([128, JT], F32, tag="s1")
            s2 = micro.tile([128, JT], F32, tag="s2")
            s3 = micro.tile([128, JT], F32, tag="s3")
            for j in range(JT):
                tew = scratch.tile([128, D], BF16, tag="scr", name="tew%d" % j)
                nc.gpsimd.tensor_tensor(
                    out=tew[:], in0=ebl[:, j], in1=wll[:, j],
                    op=mybir.AluOpType.add,
                )
                for si, (src, acc) in enumerate(
                    ((ebl[:, j], s1), (wll[:, j], s2), (tew[:], s3))
                ):
                    scr = scratch.tile([128, D], BF16, tag="scr",
                                       name="sq%d_%d" % (j, si))
                    nc.scalar.activation(
                        scr[:], src, mybir.ActivationFunctionType.Square,
                        accum_out=acc[:, j:j + 1],
                    )
            # d2 = 2*(e.w);  g = d2 * S / (2*|e|*|w|) = S*cos
            d2 = micro.tile([128, JT], F32, tag="d2")
            nc.gpsimd.tensor_tensor(
                out=d2[:], in0=s3[:], in1=s1[:], op=mybir.AluOpType.subtract
            )
            nc.gpsimd.tensor_tensor(
                out=d2[:], in0=d2[:], in1=s2[:], op=mybir.AluOpType.subtract
            )
            pr = micro.tile([128, JT], F32, tag="pr")
            nc.gpsimd.tensor_tensor(
                out=pr[:], in0=s1[:], in1=s2[:], op=mybir.AluOpType.mult
            )
            rt = micro.tile([128, JT], F32, tag="rt")
            nc.scalar.activation(
                rt[:], pr[:], mybir.ActivationFunctionType.Sqrt,
                scale=4.0 / (S * S),
            )

            # ---------------- epilogue + output per m-tile ----------------
            def emit_epilogue(m, psC, split=False):
                ot = outp.tile([128, W], BF16, tag="ot", name="ot%d" % m)
                chunks = NCHUNKS if split else [(0, W)]
                for (c0, cn) in chunks:
                    nc.vector.scalar_tensor_tensor(
                        out=ot[:, c0:c0 + cn], in0=psC[:, c0:c0 + cn],
                        scalar=rs_em[:, m:m + 1], in1=ws_bc[:, c0:c0 + cn],
                        op0=mybir.AluOpType.mult, op1=mybir.AluOpType.mult,
                    )
                    nc.sync.dma_start(
                        out_h[m * 128:(m + 1) * 128, c0:c0 + cn],
                        ot[:, c0:c0 + cn],
                    )

            def emit_fixup_part2():
                nc.vector.reciprocal_approx_fast(out=rt[:], in_=rt[:])
                g = micro.tile([128, JT], F32, tag="g")
                nc.gpsimd.tensor_tensor(
                    out=g[:], in0=d2[:], in1=rt[:], op=mybir.AluOpType.mult
                )
                fixv_sb = persist.tile([128, JT], F32, tag="fixv_sb")
                _emit_margin_math(nc, micro, g, fixv_sb)
                # separate queue: never blocks the out-tile DMA stream
                nc.gpsimd.dma_start(
                    out=fixv_h.rearrange("(j p) -> p j", p=128), in_=fixv_sb[:]
                )

            emit_epilogue(0, psC0)
            emit_epilogue(1, psC1)
            for m in range(2, MT):
                pss = emit_mms(m)
                emit_epilogue(m, pss, split=(m == MT - 1))
                if m == 5:
                    emit_fixup_part2()

    nc.finalize()
    return nc


_NC_CACHE: bass.Bass | None = None


def get_nc() -> bass.Bass:
    global _NC_CACHE
    if _NC_CACHE is None:
        _NC_CACHE = build_nc()
    return _NC_CACHE


def make_in_maps(embedding: np.ndarray, kernel: np.ndarray, labels: np.ndarray):
    embedding = np.asarray(embedding, dtype=np.float32)
    kernel = np.asarray(kernel, dtype=np.float32)
    labels = np.asarray(labels, dtype=np.int64)

    import ml_dtypes

    embT = np.ascontiguousarray(embedding.T).astype(ml_dtypes.bfloat16)
    kern_pad = np.ones((D, NCORES * SHARD), dtype=np.float32)
    kern_pad[:, :C] = kernel
    kernT = np.ascontiguousarray(kernel.T)  # [C, D]

    in_maps = []
    for i in range(NCORES):
        wi = np.ascontiguousarray(
            kern_pad[:, i * SHARD:(i + 1) * SHARD]
        ).astype(ml_dtypes.bfloat16)
        sl = slice(i * BS, (i + 1) * BS)
        embs = np.ascontiguousarray(embedding[sl]).astype(ml_dtypes.bfloat16)
        wlt = np.ascontiguousarray(kernT[labels[sl]]).astype(ml_dtypes.bfloat16)
        in_maps.append({"embT": embT, "w": wi, "embs": embs, "wlt": wlt})
    return in_maps


def assemble(results, labels) -> np.ndarray:
    full = np.concatenate(
        [np.asarray(results[i]["out"], dtype=np.float32) for i in range(NCORES)],
        axis=1,
    )[:, :C]
    labels = np.asarray(labels, dtype=np.int64)
    fixv = np.concatenate(
        [np.asarray(results[i]["fixv"], dtype=np.float32) for i in range(NCORES)]
    )
    full[np.arange(B), labels] = fixv
    return full


def kernel(embedding: np.ndarray, kernel: np.ndarray, labels: np.ndarray) -> np.ndarray:
    nc = get_nc()
    in_maps = make_in_maps(embedding, kernel, labels)
    last_err = None
    for _attempt in range(3):
        try:
            res = run_bass_kernel_spmd(nc, in_maps, core_ids=list(range(NCORES)))
            return assemble(res.results, labels)
        except Exception as e:  # transient NRT/device errors: retry
            last_err = e
    raise last_err


if __name__ == "__main__":
    rng = np.random.default_rng(0)
    emb = rng.standard_normal((B, D), dtype=np.float32)
    kern = (rng.standard_normal((D, C), dtype=np.float32) * 0.05).astype(np.float32)
    labs = rng.integers(0, C, size=(B,), dtype=np.int32)
    out = kernel(emb, kern, labs)
    print(out.shape, out.dtype)
